# revision 38
# baseline (speedup 1.0000x reference)
"""2-layer multi-head GAT on 8 Trainium2 NeuronCores (fp16, v4).

Sharding: nodes assigned to (core, 128-wide dst block) by LPT on in-degree
(even per-block edge counts across cores -> minimal tile padding; 6272
padded slots/core). Edges live on their dst's core, sorted by dst block,
and split by src row-half (A: first 26 blocks of slots, B: rest) so gather
indices fit int16 and AllGathers pipeline with edge compute.

Per layer:
  1. per-core GEMM  feat|el|er = h @ [W | W@Al | W@Ar]  (fp16 PE, 1cyc/row).
     feat columns stored (j-major, h-minor) interleaved so the later
     alpha-broadcast multiply is a packed-last-dim DVE op (2x fp16 mode).
     er stays in SBUF (never round-trips DRAM).
  2. AllGather A-half after GEMM blocks 0-25, B-half after 26-48 (fp16 rows
     of 768B). Phase-A edge processing needs only table A, so AG(B) hides
     under it; layer-2 GEMM is interleaved into layer-1 phase-B flushes and
     AG2(A)/AG2(B) are issued at the tail of that phase (mid-phase issue at
     block 25 hard-faults the device; see KAGPOS).
  3. per dst-block, per 128-edge tile: dma_gather of src rows (768B rows,
     4-block-merged calls split across the 4 SWDGE queues; queue choice
     comes from a two-pass build (_build_mq) that pins each Tile DMASW
     sem lane to one queue — naive queue choice races the sems and
     intermittently hard-faults the device),
     selection matrix MT via per-tile tensor_scalar is_equal (4x DVE mode),
     er[dst] per edge via PE transpose of MT + tiny matmul against the
     SBUF-resident er (no 256B/edge er gather), e-chain e=lrelu(el+er),
     X=exp(e) written straight into W's denominator columns by the ACT
     engine, W=feat*X (packed 2x DVE), aggregation matmuls in PSUM (fp16).
  4. phase A stashes partial sums in SBUF; phase B combines, divides by the
     softmax denominator, applies ELU, DMA-transposes fb for the next GEMM.

Env gates (debug/experiments, defaults are the shipping config): KSIM
(1-core no-collective build for CoreSim), KAG=0 (skip collectives),
KAGPOS (collective placement), KEDGE=0/KGOFF=1 (timing isolation),
KFP8 (fp8e3m4 feature tables: 1=both layers [rel err 2.7e-2, fails],
2=layer-1 only [1.79e-2, passes but thin margin]; default 0 = fp16,
rel err 1.5e-3).
"""
import sys
sys.path.insert(0, '/opt/trn_rl_repo')
import numpy as np

N_NODES = 50000
N_EDGES = 800000
IN_DIM = 256
HID = 64
HEADS = 4
NEG_SLOPE = 0.2
N_CORES = 8
NPC = N_NODES // N_CORES          # 6250 real nodes per core
P = 128
NB = 49                            # dst blocks per core
NPAD = NB * P                      # 6272 padded nodes per core
NBA = 26                           # blocks whose rows live in table A
A_ROWS = NBA * P                   # 3200 local rows in table A
B_ROWS = (NB - NBA) * P            # 3072 local rows in table B
RA = N_CORES * A_ROWS              # 25600
RB = N_CORES * B_ROWS              # 24576
ESL = 384                          # fp16 table row elems (768B)
CG = 260                           # feat + denom columns in W
QUAD = 4                           # blocks per merged gather call
PAD_LDST = 999.0

# feature interleave: standard col c = h*64+j  <->  stored col j*4+h
_PERM = np.arange(IN_DIM).reshape(HEADS, HID).T.reshape(-1)   # perm[j*4+h] = h*64+j


def _wrap_idx(idx_list):
    """[n] int -> [128, n//16] int16 wrapped-in-16 layout, replicated."""
    n = len(idx_list)
    assert n % 16 == 0
    arr = np.asarray(idx_list, np.int16).reshape(n // 16, 16)  # [s, q]
    w16 = arr.T                                                # [16, s]
    return np.tile(w16, (8, 1))                                # [128, s]


_PLACE = {}


def _placement(dst):
    """LPT in-degree balancing: node -> (core, padded slot) so per-(core,
    block) edge counts are even across cores (less tile padding)."""
    import heapq
    deg = np.bincount(dst, minlength=N_NODES)
    order = np.argsort(-deg, kind="stable")
    NBINS = N_CORES * NB
    heap = [(0, b) for b in range(NBINS)]
    heapq.heapify(heap)
    fill = np.zeros(NBINS, np.int64)
    core_of_n = np.zeros(N_NODES, np.int32)
    slot_of_n = np.zeros(N_NODES, np.int32)
    for n in order:
        while True:
            load, b = heapq.heappop(heap)
            if fill[b] < P:
                break
        core_of_n[n] = b // NB
        slot_of_n[n] = (b % NB) * P + fill[b]
        fill[b] += 1
        heapq.heappush(heap, (load + deg[n], b))
    return core_of_n, slot_of_n


def _prep(x, src, dst, W1, al1, ar1, W2, al2, ar2, kdt=16):
    src = np.asarray(src).astype(np.int64)
    dst = np.asarray(dst).astype(np.int64)
    x = np.asarray(x, np.float32)

    core_of_n, slot_of_n = _placement(dst)
    _PLACE["core"] = core_of_n
    _PLACE["slot"] = slot_of_n

    own = core_of_n[src]
    loc = slot_of_n[src]
    in_a = loc < A_ROWS
    rowA = own.astype(np.int64) * A_ROWS + loc        # valid where in_a
    rowB = own.astype(np.int64) * B_ROWS + (loc - A_ROWS)  # valid where ~in_a

    core_of = core_of_n[dst]
    ld_all = slot_of_n[dst]
    blk_all = ld_all // P
    lin_all = ld_all % P

    eA = [[[] for _ in range(NB)] for _ in range(N_CORES)]
    eB = [[[] for _ in range(NB)] for _ in range(N_CORES)]
    order = np.lexsort((src, dst))
    for e in order:
        c = core_of[e]
        b = blk_all[e]
        (eA if in_a[e] else eB)[c][b].append(e)

    T_A = [max(1, -(-max(len(eA[c][b]) for c in range(N_CORES)) // P)) for b in range(NB)]
    T_B = [max(1, -(-max(len(eB[c][b]) for c in range(N_CORES)) // P)) for b in range(NB)]
    for b in range(NB):
        if all(len(eA[c][b]) == 0 for c in range(N_CORES)):
            T_A[b] = 0
        if all(len(eB[c][b]) == 0 for c in range(N_CORES)):
            T_B[b] = 0

    plan = {"T_A": T_A, "T_B": T_B}

    # attention projection matrices (per-head block diagonal)
    def head_mat(a):
        m = np.zeros((IN_DIM, HEADS), np.float64)
        a = np.asarray(a, np.float64)
        for h in range(HEADS):
            m[h * HID:(h + 1) * HID, h] = a[h]
        return m

    def wext(W, al, ar, row_perm):
        """[256, 264] = [W(cols interleaved) | W@Al | W@Ar], rows optionally
        permuted (for layer 2 whose input features are interleaved)."""
        W = np.asarray(W, np.float64)
        m = np.concatenate([W[:, _PERM], W @ head_mat(al), W @ head_mat(ar)], axis=1)
        if row_perm is not None:
            m = m[row_perm]
        out = np.zeros((P, 2 * 264), np.float16)
        for g in range(2):
            out[:, g * 264:(g + 1) * 264] = m[g * P:(g + 1) * P].astype(np.float16)
        return out

    W1k = wext(W1, al1, ar1, None)
    W2k = wext(W2, al2, ar2, _PERM)
    iota = np.tile(np.arange(P, dtype=np.float16), (P, 1))
    ident = np.eye(P, dtype=np.float16)

    in_maps = []
    for c in range(N_CORES):
        xl = np.zeros((NPAD, IN_DIM), np.float32)
        mine = core_of_n == c
        xl[slot_of_n[mine]] = x[mine]
        xT = np.zeros((P, 2 * NPAD), np.float16)
        for g in range(2):
            xT[:, g * NPAD:(g + 1) * NPAD] = xl[:, g * P:(g + 1) * P].T.astype(np.float16)

        def build_phase(elists, rows, T):
            idx_cols = []
            ldst_cols = np.full((P, max(sum(T), 1)), PAD_LDST, np.float32)
            toff = 0
            for b in range(NB):
                el = elists[c][b]
                n = T[b] * P
                if n == 0:
                    continue
                ii = [int(rows[e]) for e in el] + [0] * (n - len(el))
                lv = ([float(lin_all[e]) for e in el]
                      + [PAD_LDST] * (n - len(el)))
                idx_cols.append(_wrap_idx(ii))
                ldst_cols[:, toff:toff + T[b]] = \
                    np.asarray(lv, np.float32).reshape(T[b], P).T
                toff += T[b]
            idx = (np.concatenate(idx_cols, axis=1) if idx_cols
                   else np.zeros((P, 8), np.int16))
            return idx, ldst_cols

        idxA, ldstA = build_phase(eA, rowA, T_A)
        idxB, ldstB = build_phase(eB, rowB, T_B)

        in_maps.append({
            "xT": xT, "W1k": W1k, "W2k": W2k,
            "idxA": idxA, "idxB": idxB,
            "ldstA": ldstA, "ldstB": ldstB,
            "iota": iota, "ident": ident,
        })

    plan["idxA_cols"] = in_maps[0]["idxA"].shape[1]
    plan["idxB_cols"] = in_maps[0]["idxB"].shape[1]
    plan["ldstA_cols"] = in_maps[0]["ldstA"].shape[1]
    plan["ldstB_cols"] = in_maps[0]["ldstB"].shape[1]
    return in_maps, plan


def _build(plan, qmap=None):
    import os
    KSIM = int(os.environ.get("KSIM", "0"))
    KAG = int(os.environ.get("KAG", "1"))
    KAGPOS = int(os.environ.get("KAGPOS", "2"))
    KEDGE = int(os.environ.get("KEDGE", "1"))
    KGOFF = int(os.environ.get("KGOFF", "0"))
    KFP8 = int(os.environ.get("KFP8", "0"))
    import concourse.bass as bass
    import concourse.bacc as bacc
    import concourse.mybir as mybir
    import concourse.tile as tile

    dt = mybir.dt
    F16 = dt.float16
    # per-layer table dtype: fp8 rows (512B) where tolerable, fp16 (768B) else
    # KFP8: 0 = fp16 both, 1 = fp8 both, 2 = fp8 layer1 / fp16 layer2
    FDTs = {1: dt.float8e3 if KFP8 in (1, 2) else F16,
            2: dt.float8e3 if KFP8 == 1 else F16}
    ESLs = {1: 512 if KFP8 in (1, 2) else ESL,
            2: 512 if KFP8 == 1 else ESL}
    T_A, T_B = plan["T_A"], plan["T_B"]

    nc = bacc.Bacc("TRN2", target_bir_lowering=False, debug=False,
                   num_devices=(1 if KSIM else N_CORES),
                   num_swdge_queues=4)
    xT_ap = nc.dram_tensor("xT", [P, 2 * NPAD], F16, kind="ExternalInput").ap()
    W1k_ap = nc.dram_tensor("W1k", [P, 2 * 264], F16, kind="ExternalInput").ap()
    W2k_ap = nc.dram_tensor("W2k", [P, 2 * 264], F16, kind="ExternalInput").ap()
    idxA_ap = nc.dram_tensor("idxA", [P, plan["idxA_cols"]], dt.int16, kind="ExternalInput").ap()
    idxB_ap = nc.dram_tensor("idxB", [P, plan["idxB_cols"]], dt.int16, kind="ExternalInput").ap()
    ldstA_ap = nc.dram_tensor("ldstA", [P, plan["ldstA_cols"]], dt.float32, kind="ExternalInput").ap()
    ldstB_ap = nc.dram_tensor("ldstB", [P, plan["ldstB_cols"]], dt.float32, kind="ExternalInput").ap()
    iota_ap = nc.dram_tensor("iota", [P, P], F16, kind="ExternalInput").ap()
    ident_ap = nc.dram_tensor("ident", [P, P], F16, kind="ExternalInput").ap()
    out_ap = nc.dram_tensor("out", [NPAD, IN_DIM], dt.float32, kind="ExternalOutput").ap()

    AF = mybir.ActivationFunctionType
    ALU = mybir.AluOpType
    gather_handles = []

    with tile.TileContext(nc) as tc:
        with tc.tile_pool(name="const", bufs=1) as cpool, \
             tc.tile_pool(name="gemm", bufs=2) as gpool, \
             tc.tile_pool(name="edge", bufs=2) as epool, \
             tc.tile_pool(name="flush", bufs=2) as fpool, \
             tc.tile_pool(name="psum", bufs=2, space="PSUM") as pp, \
             tc.tile_pool(name="dram", bufs=1, space="DRAM") as dram:

            iota_t = cpool.tile([P, P], F16)
            ident_t = cpool.tile([P, P], F16)
            idxA_t = cpool.tile([P, plan["idxA_cols"]], dt.int16)
            idxB_t = cpool.tile([P, plan["idxB_cols"]], dt.int16)
            ldstA_t = cpool.tile([P, plan["ldstA_cols"]], dt.float32)
            ldstB_t = cpool.tile([P, plan["ldstB_cols"]], dt.float32)
            w1_t = cpool.tile([P, 2 * 264], F16)
            w2_t = cpool.tile([P, 2 * 264], F16)
            nc.sync.dma_start(iota_t[:], iota_ap[:])
            nc.sync.dma_start(ident_t[:], ident_ap[:])
            nc.sync.dma_start(idxA_t[:], idxA_ap[:])
            nc.sync.dma_start(idxB_t[:], idxB_ap[:])
            nc.sync.dma_start(ldstA_t[:], ldstA_ap[:])
            nc.sync.dma_start(ldstB_t[:], ldstB_ap[:])
            nc.sync.dma_start(w1_t[:], W1k_ap[:])
            nc.sync.dma_start(w2_t[:], W2k_ap[:])

            gctr = [0]   # gather emission counter (indexes qmap)
            # per-layer er values [dst-lane, 4], SBUF resident
            er_all = [cpool.tile([P, NB * HEADS], F16, name=f"er_all{i}")
                      for i in range(2)]
            # phase-A partial aggregation stash
            stash = cpool.tile([P, NB * CG], F16)

            _ashared = "Local" if KSIM else "Shared"
            tabA_loc = {L: dram.tile([A_ROWS, ESLs[L]], FDTs[L], name=f"tabA_loc{L}")
                        for L in (1, 2)}
            tabB_loc = {L: dram.tile([B_ROWS, ESLs[L]], FDTs[L], name=f"tabB_loc{L}")
                        for L in (1, 2)}
            tabA1 = dram.tile([RA, ESLs[1]], FDTs[1], addr_space=_ashared)
            tabB1 = dram.tile([RB, ESLs[1]], FDTs[1], addr_space=_ashared)
            tabA2 = dram.tile([RA, ESLs[2]], FDTs[2], addr_space=_ashared)
            tabB2 = dram.tile([RB, ESLs[2]], FDTs[2], addr_space=_ashared)
            h1T = dram.tile([P, 2 * NPAD], F16)

            def gemm_quad(layer, b0, nblk):
                wk = w1_t if layer == 1 else w2_t
                src = xT_ap if layer == 1 else h1T
                hkq = []
                for g in range(2):
                    hk = gpool.tile([P, nblk * P], F16, name=f"hkq{g}", tag=f"hkq{g}")
                    nc.sync.dma_start(hk[:], src[:, g * NPAD + b0 * P:
                                                 g * NPAD + (b0 + nblk) * P])
                    hkq.append(hk)
                for bi in range(nblk):
                    gemm_block(layer, b0 + bi, hkq, bi)

            def gemm_block(layer, b, hkq, bi):
                wk = w1_t if layer == 1 else w2_t
                ps = pp.tile([P, 264], dt.float32, space="PSUM", name="gemm_ps", tag="gemm_ps")
                for g in range(2):
                    nc.tensor.matmul(out=ps[:], lhsT=hkq[g][:, bi * P:(bi + 1) * P],
                                     rhs=wk[:, g * 264:(g + 1) * 264],
                                     start=(g == 0), stop=(g == 1))
                fdt, esl = FDTs[layer], ESLs[layer]
                sb = gpool.tile([P, esl], fdt, name=f"gemm_sb{layer}",
                                tag=f"gemm_sb{layer}")
                if fdt != F16:
                    nc.scalar.activation(sb[:, 0:256], ps[:, 0:256], AF.Copy)
                    nc.scalar.activation(sb[:].bitcast(F16)[:, 128:132],
                                         ps[:, 256:260], AF.Copy)
                    nc.vector.memset(sb[:, 264:esl], 0.0)
                else:
                    nc.scalar.activation(sb[:, 0:CG], ps[:, 0:CG], AF.Copy)
                    nc.vector.memset(sb[:, CG:esl], 0.0)
                nc.vector.tensor_copy(er_all[layer - 1][:, b * 4:(b + 1) * 4], ps[:, 260:264])
                tab_loc = tabA_loc[layer] if b < NBA else tabB_loc[layer]
                r0 = b * P if b < NBA else (b - NBA) * P
                nc.sync.dma_start(tab_loc[r0:r0 + P, :], sb[:])

            def edge_phase(layer, phase):
                T = T_A if phase == 0 else T_B
                idx_t = idxA_t if phase == 0 else idxB_t
                ldst_t = ldstA_t if phase == 0 else ldstB_t
                if phase == 0:
                    tab = tabA1 if layer == 1 else tabA2
                else:
                    tab = tabB1 if layer == 1 else tabB2
                erl = er_all[layer - 1]
                fdt, esl = FDTs[layer], ESLs[layer]

                # quad-merged gathers
                toff = 0  # tile offset within this phase
                for q0 in range(0, NB, QUAD):
                    blocks = [b for b in range(q0, min(q0 + QUAD, NB)) if T[b] > 0]
                    tq = sum(T[b] for b in blocks)
                    if tq == 0:
                        for b in range(q0, min(q0 + QUAD, NB)):
                            finish_block(layer, phase, b, None, None)
                        continue
                    if KGOFF:
                        G = None
                    else:
                        G = epool.tile([P, tq * esl], fdt, name=f"G{layer}",
                                       tag="G", bufs=4)
                        # Split each quad across the 4 SWDGE queues for
                        # gather BW. Tile assigns DMASW sem lanes round-robin
                        # (idx % 8) over Pool DMA insts in program order, and
                        # a sem must only ever be updated from ONE queue
                        # (cross-queue updates race -> device faults). Using
                        # queue = (global gather count) % 4 pins sem lane s
                        # to queue s % 4 permanently, which is safe.
                        bnds = [round(si * tq / 4) for si in range(5)]
                        for si in range(4):
                            t0s, t1s = bnds[si], bnds[si + 1]
                            if t1s == t0s:
                                continue
                            gi = gctr[0]
                            gctr[0] += 1
                            qn = qmap[gi] if qmap is not None else 0
                            gh = nc.gpsimd.dma_gather(
                                out_ap=G[:, t0s * esl:t1s * esl]
                                    .rearrange("p (t e) -> p t e", e=esl),
                                in_ap=tab[:],
                                idxs_ap=idx_t[:, 8 * (toff + t0s): 8 * (toff + t1s)],
                                num_idxs=(t1s - t0s) * P,
                                num_idxs_reg=(t1s - t0s) * P, elem_size=esl,
                                single_packet=False, queue_num=qn)
                            gather_handles.append(gh)
                    goff = 0  # tile offset within G
                    for b in range(q0, min(q0 + QUAD, NB)):
                        t = T[b]
                        if t == 0 or not KEDGE or G is None:
                            finish_block(layer, phase, b, None, None)
                            continue
                        process_block(layer, phase, b, t, G, goff, ldst_t,
                                      toff + goff, erl, fdt, esl)
                        goff += t
                    toff += tq

            def process_block(layer, phase, b, t, G, goff, ldst_t, loff, erl,
                              fdt, esl):
                # selection matrix MT[e, d] = (d == ldst[e])
                MT = epool.tile([P, t * P], F16, name="MT", tag="MT", bufs=3)
                for ti in range(t):
                    nc.vector.tensor_scalar(
                        out=MT[:, ti * P:(ti + 1) * P], in0=iota_t[:],
                        scalar1=ldst_t[:, loff + ti: loff + ti + 1], scalar2=None,
                        op0=ALU.is_equal)
                # MT2 = MT^T per tile (PE transpose, 8 tiles per 2KB PSUM bank)
                MT2 = epool.tile([P, t * P], F16, name="MT2", tag="MT2", bufs=3)
                for t0 in range(0, t, 8):
                    n8 = min(8, t - t0)
                    trp = pp.tile([P, 8 * P], F16, space="PSUM", name="tr_ps", tag="tr_ps")
                    for k in range(n8):
                        nc.tensor.transpose(out=trp[:, k * P:(k + 1) * P],
                                            in_=MT[:, (t0 + k) * P:(t0 + k + 1) * P],
                                            identity=ident_t[:])
                    nc.scalar.activation(MT2[:, t0 * P:(t0 + n8) * P],
                                         trp[:, 0:n8 * P], AF.Copy)
                # er per edge: ER[e, h] = sum_c MT2[c, e] * er[c, h]
                er_ps = pp.tile([P, t * 4], dt.float32, space="PSUM", name="er_ps", tag="er_ps", bufs=1)
                for ti in range(t):
                    nc.tensor.matmul(out=er_ps[:, ti * 4:(ti + 1) * 4],
                                     lhsT=MT2[:, ti * P:(ti + 1) * P],
                                     rhs=erl[:, b * 4:(b + 1) * 4],
                                     start=True, stop=True, skip_group_check=True)
                # e-chain
                g3 = G[:, goff * esl:(goff + t) * esl].rearrange("p (t c) -> p t c", c=esl)
                if fdt != F16:
                    c2 = esl // 2
                    gel = G[:].bitcast(F16)[:, goff * c2:(goff + t) * c2] \
                              .rearrange("p (t c) -> p t c", c=c2)[:, :, 128:132]
                else:
                    gel = g3[:, :, 256:260]
                E = epool.tile([P, t * 4], dt.float32, name="E", tag="E", bufs=3)
                e3 = E[:].rearrange("p (t h) -> p t h", h=4)
                nc.vector.tensor_tensor(out=e3, in0=gel,
                                        in1=er_ps[:].rearrange("p (t h) -> p t h", h=4),
                                        op=ALU.add)
                L = epool.tile([P, t * 4], dt.float32, name="L", tag="L", bufs=3)
                nc.vector.tensor_scalar_mul(L[:], E[:], NEG_SLOPE)
                nc.vector.tensor_tensor(out=L[:], in0=E[:], in1=L[:], op=ALU.max)
                # W = [feat * X | X], X written straight into cols 256:260 by ACT
                W = epool.tile([P, t * CG], F16, name="W", tag="W", bufs=3)
                w3 = W[:].rearrange("p (t c) -> p t c", c=CG)
                nc.scalar.activation(w3[:, :, 256:260],
                                     L[:].rearrange("p (t h) -> p t h", h=4), AF.Exp)
                w4 = w3[:, :, 0:256].rearrange("p t (j h) -> p t j h", h=4)
                gf4 = g3[:, :, 0:256].rearrange("p t (j h) -> p t j h", h=4)
                x4 = w3[:, :, 256:260].rearrange("p t (o h) -> p t o h", o=1) \
                                      .to_broadcast([P, t, 64, 4])
                nc.vector.tensor_tensor(out=w4, in0=gf4, in1=x4, op=ALU.mult)
                # aggregate
                agg = pp.tile([P, CG], dt.float32, space="PSUM", name="agg_ps", tag="agg_ps")
                for ti in range(t):
                    nc.tensor.matmul(out=agg[:], lhsT=MT[:, ti * P:(ti + 1) * P],
                                     rhs=W[:, ti * CG:(ti + 1) * CG],
                                     start=(ti == 0), stop=(ti == t - 1))
                finish_block(layer, phase, b, agg, None)

            def finish_block(layer, phase, b, agg, _unused):
                if phase == 0:
                    # stash phase-A partials (or zeros if no A edges)
                    if agg is None:
                        nc.vector.memset(stash[:, b * CG:(b + 1) * CG], 0.0)
                    else:
                        nc.scalar.activation(stash[:, b * CG:(b + 1) * CG],
                                             agg[:], AF.Copy)
                    return
                # phase B: combine + softmax divide + ELU
                comb = fpool.tile([P, CG], dt.float32, name="comb", tag="comb")
                if agg is None:
                    nc.vector.tensor_copy(comb[:], stash[:, b * CG:(b + 1) * CG])
                else:
                    nc.vector.tensor_tensor(out=comb[:], in0=stash[:, b * CG:(b + 1) * CG],
                                            in1=agg[:], op=ALU.add)
                dmx = fpool.tile([P, 4], dt.float32, name="dmx", tag="dmx")
                nc.vector.tensor_scalar_max(dmx[:], comb[:, 256:260], 1e-30)
                rec = fpool.tile([P, 4], dt.float32, name="rec", tag="rec")
                nc.vector.reciprocal(rec[:], dmx[:])
                ob = fpool.tile([P, 256], dt.float32, name="ob", tag="ob")
                ob4 = ob[:].rearrange("p (j h) -> p j h", h=4)
                rec4 = rec[:].rearrange("p (o h) -> p o h", o=1).to_broadcast([P, 64, 4])
                nc.vector.tensor_tensor(out=ob4,
                                        in0=comb[:, 0:256].rearrange("p (j h) -> p j h", h=4),
                                        in1=rec4, op=ALU.mult)
                # ELU: relu(x) + exp(min(x,0)) - 1
                nb_t = fpool.tile([P, 256], dt.float32, name="nb", tag="nb")
                nc.vector.tensor_scalar_min(nb_t[:], ob[:], 0.0)
                en = fpool.tile([P, 256], dt.float32, name="en", tag="en")
                nc.scalar.activation(en[:], nb_t[:], AF.Exp)
                pb = fpool.tile([P, 256], dt.float32, name="pb", tag="pb")
                nc.scalar.activation(pb[:], ob[:], AF.Relu)
                if layer == 1:
                    fb = fpool.tile([P, 256], F16, name="fb", tag="fb")
                    nc.vector.tensor_tensor(out=fb[:], in0=en[:], in1=pb[:], op=ALU.add)
                    nc.vector.tensor_scalar_add(fb[:], fb[:], -1.0)
                    for g in range(2):
                        tsb = fpool.tile([P, P], F16, name="tsb", tag="tsb")
                        nc.sync.dma_start(tsb[:], fb[:, g * P:(g + 1) * P],
                                          transpose=True)
                        nc.sync.dma_start(h1T[:, g * NPAD + b * P: g * NPAD + (b + 1) * P], tsb[:])
                    # layer-2 GEMM once a quad of h1 blocks has landed
                    if b + 1 in (NBA, NB) or (b + 1) % 4 == 0:
                        q0 = (b // 4) * 4
                        if q0 < NBA <= b:
                            q0 = NBA
                        gemm_quad(2, q0, b + 1 - q0)
                    if KAGPOS:
                        agb = NBA - 1 if KAGPOS == 1 else (NB - 2 if KAGPOS == 2 else NB - 1)
                        if b == agb:
                            ag(tabA_loc[2], tabA2)
                        elif b == NB - 1:
                            ag(tabB_loc[2], tabB2)
                else:
                    fb = fpool.tile([P, 256], dt.float32, name="fb32", tag="fb32")
                    nc.vector.tensor_tensor(out=fb[:], in0=en[:], in1=pb[:], op=ALU.add)
                    nc.vector.tensor_scalar_add(fb[:], fb[:], -1.0)
                    nc.sync.dma_start(out_ap[b * P:(b + 1) * P, :], fb[:])

            def ag(src_tile, dst_tile):
                if KSIM or not KAG:
                    return
                nc.gpsimd.collective_compute(
                    "AllGather", mybir.AluOpType.bypass,
                    replica_groups=[list(range(N_CORES))],
                    ins=[src_tile.opt()],
                    outs=[dst_tile.opt()])

            # ---- schedule ----
            for b0 in range(0, NBA, 4):
                gemm_quad(1, b0, min(4, NBA - b0))
            ag(tabA_loc[1], tabA1)
            for b0 in range(NBA, NB, 4):
                gemm_quad(1, b0, min(4, NB - b0))
            ag(tabB_loc[1], tabB1)
            edge_phase(1, 0)
            edge_phase(1, 1)   # interleaves gemm_block(2, b) + AG2 launches
            if not KAGPOS:
                ag(tabA_loc[2], tabA2)
                ag(tabB_loc[2], tabB2)
            edge_phase(2, 0)
            edge_phase(2, 1)

    nc.compile()
    nc._gather_handles = gather_handles
    return nc


def _gather_lanes(nc):
    """Per emitted gather: the Tile-assigned DMASW sem lane (0..7), in
    emission order (instruction names are minted in emission order).
    DMASW lanes are procs 11..18."""
    found = []
    for blk in nc.m.functions[0].blocks:
        for inst in blk.instructions:
            if type(inst).__name__ == "InstDMAGatherAnt":
                try:
                    order = int(str(inst.name).split("-")[-1])
                    proc = int(inst.bass_scheduled_proc) - 11
                except (TypeError, ValueError):
                    return None
                found.append((order, proc))
    found.sort()
    lanes = [p for _, p in found]
    return lanes if lanes else None


def _build_mq(plan):
    """Two-pass build: learn each gather's DMASW sem lane, then rebuild with
    queue = lane % 4 so every sem lane is pinned to one SWDGE queue (cross-
    queue sem updates race on HW). Falls back to single-queue if the
    schedule does not reach a fixpoint."""
    nc = _build(plan, qmap=None)
    for _ in range(3):
        lanes = _gather_lanes(nc)
        if lanes is None or any(not (0 <= l < 8) for l in lanes):
            return _build(plan, qmap=None)
        qmap = [l % 4 for l in lanes]
        nc2 = _build(plan, qmap=qmap)
        if _gather_lanes(nc2) == lanes:
            return nc2          # aligned: lane l always served by queue l%4
        nc = nc2
    return _build(plan, qmap=None)


def _finish(results):
    """Per-core 'out' [NPAD, 256] (cols interleaved j*4+h, rows placed by
    _placement) -> tuple of heads."""
    outs = np.stack([np.asarray(results[c]["out"]) for c in range(N_CORES)])
    h = outs[_PLACE["core"], _PLACE["slot"]]                # [N, 256]
    h = h.reshape(N_NODES, HID, HEADS).transpose(0, 2, 1)   # [N, H, D]
    return tuple(h[:, i] for i in range(HEADS))


def kernel(**inputs):
    from concourse.bass_utils import run_bass_kernel_spmd
    in_maps, plan = _prep(inputs["x"], inputs["src"], inputs["dst"],
                          inputs["W1"], inputs["al1"], inputs["ar1"],
                          inputs["W2"], inputs["al2"], inputs["ar2"])
    nc = _build_mq(plan)
    res = run_bass_kernel_spmd(nc, in_maps, core_ids=list(range(N_CORES)),
                               trace=False)
    return _finish(res.results)
